# revision 12
# baseline (speedup 1.0000x reference)
"""Trainium2 Bass kernel for BEVLayerInjector (8-core SPMD), v3.

Sharding: data-parallel over batch B=4 x 2-way split of the NV=1024 gathered
vision tokens -> 8 shards, one per NeuronCore. The gather (hidden_states ->
vis) and the final scatter + residual add run on the host; the device computes
delta = MLP3(LN(vh + CrossAttn(vh, bh) @ Wo^T)) per shard.

v3 changes vs v2 (which measured ~184us/iter, phase C alone 85us):
  - phase C processes heads in PAIRS: even head on PE rows 0:63, odd head on
    rows 64:127.  Score matmuls for the two heads are emitted interleaved so
    the 64-row subarray groups run concurrently (~2x PE throughput on the
    K=64 score matmuls), and each k-tile's pair of score blocks is drained by
    ONE ACT exp call [128,1024] (ACT is the phase-C bottleneck engine).
  - softmax normalization: per pair ONE DVE reciprocal [1,1024] on the
    ones-rider denominator rows, gpsimd partition_broadcast to 64 rows (was:
    PE broadcast matmul + DVE copy), and ONE DVE multiply for both heads.
    Removes 12 PE matmuls and shortens the per-head cross-engine chain.
  - ao accumulates per pair in one 2-bank psX tile (even head bank a, odd
    head bank b) - no extra PSUM pools.
  - LayerNorm: stats ones are pre-scaled by 1/DIM so mean/E[x^2] come out of
    PSUM directly; var fused into one scalar_tensor_tensor; mean/rstd
    broadcast matmuls write both banks so the final normalize is 4 batched
    DVE ops instead of 8.
  - dead wk/wv copies dropped from the packed bf16 weight bundle (wsm now
    holds only w1b, w2b, wq, w3a).

Layout: "feature-major" tensors keep the contraction dim on SBUF partitions
([feature, token]); weights are shipped pre-transposed as [in, out] and
k-tiled as [ktile, 128, out] so every DMA is contiguous.
"""

import functools
import os
import sys

sys.path.insert(0, "/opt/trn_rl_repo")

import numpy as np
import ml_dtypes

B, S, HID = 4, 4096, 3584
BEV, DIM, NH = 2048, 512, 8
NV, HW, HD = 1024, 1024, 64
EPS = 1e-5
P = 128
NCORES = 8
NVS = NV // 2              # tokens per core (512)
KT1 = HID // P             # 28 k-tiles for HID
KTD = DIM // P             # 4 k-tiles for DIM
KTB = BEV // P             # 16 k-tiles for BEV
MT = NVS // P              # 4 token m-tiles
HWT = HW // P              # 8 HW k-tiles
NO3 = HID // 512           # 7 output column chunks
NPAIR = NH // 2            # 4 head pairs

# index of each 512x512 weight (pre-transposed to [in, out]) in the packed wsm
W1B, W2B, WQ, W3A = range(4)
# fp8 copies for the DoubleRow projections
W2B8, WQ8, W3A8 = range(3)

REPS = 1  # how many times the body is emitted (timing builds use >1)
PHASES = int(os.environ.get("KPHASES", "5"))  # debug: truncate body after N phases
# fp8 DoubleRow knobs for the DIM->DIM projections (1 = fp8-DR, 0 = bf16)
K8BH = int(os.environ.get("K8BH", "1"))
K8Q = int(os.environ.get("K8Q", "1"))
K8MLP3 = int(os.environ.get("K8MLP3", "1"))

bf16 = ml_dtypes.bfloat16
f8 = ml_dtypes.float8_e4m3


def _emit(nc, tc, d, reps):
    from concourse import mybir

    dt = mybir.dt
    AF = mybir.ActivationFunctionType
    DR = mybir.MatmulPerfMode.DoubleRow

    const = tc.alloc_tile_pool(name="const", bufs=1)
    actp = tc.alloc_tile_pool(name="actp", bufs=1)
    expp = tc.alloc_tile_pool(name="expp", bufs=2)
    rcp = tc.alloc_tile_pool(name="rcp", bufs=2)
    bcp = tc.alloc_tile_pool(name="bcp", bufs=2)
    ftp = tc.alloc_tile_pool(name="ftp", bufs=2)
    outp = tc.alloc_tile_pool(name="outp", bufs=2)
    # two rotating pools of 2-bank-wide psum tiles: 2x2x2KB each = all 8 banks
    psW = tc.alloc_tile_pool(name="psW", bufs=2, space="PSUM")
    psX = tc.alloc_tile_pool(name="psX", bufs=2, space="PSUM")

    def wtile(pool):
        tag = "w" if pool is psW else "x"
        return pool.tile([P, 1024], dt.float32, name=tag, tag=tag)

    # ---------------- resident weights (loaded once, before the loop) -------
    w1aS = const.tile([P, KT1, DIM], dt.float8e4, name="w1aS", tag="w1aS")
    w2aS = const.tile([P, KTB, DIM], dt.float8e4, name="w2aS", tag="w2aS")
    w3bS = const.tile([P, KTD, HID], dt.float8e4, name="w3bS", tag="w3bS")
    wsm = const.tile([P, 4, KTD, DIM], dt.bfloat16, name="wsm", tag="wsm")
    wkv8 = const.tile([P, 2, KTD, DIM], dt.float8e4, name="wkv8", tag="wkv8")
    woF = const.tile([HD, NH, KTD, P], dt.float8e4, name="woF", tag="woF")
    for c in range(2):
        nc.sync.dma_start(
            w1aS[:, c * 14:(c + 1) * 14, :],
            d["w1aF"][c * 14:(c + 1) * 14].rearrange("a p n -> p a n"),
        )
    nc.sync.dma_start(w2aS[:], d["w2aF"].rearrange("a p n -> p a n"))
    nc.sync.dma_start(w3bS[:], d["w3bF"].rearrange("a p n -> p a n"))
    nc.sync.dma_start(wsm[:], d["wsmF"].rearrange("w a p n -> p w a n"))
    nc.sync.dma_start(wkv8[:], d["wkv8"].rearrange("w a p n -> p w a n"))
    nc.sync.dma_start(woF[:], d["woF"])

    ones_c = const.tile([P, 2], dt.bfloat16, name="ones_c", tag="ones_c")
    nc.vector.memset(ones_c[:], 1.0 / DIM)   # stats matmuls emit mean directly
    ones_r = const.tile([P, P], dt.bfloat16, name="ones_r", tag="ones_r")
    nc.vector.memset(ones_r[:], 1.0)
    eps_t = const.tile([1, 1], dt.float32, name="eps", tag="eps")
    nc.vector.memset(eps_t[:], EPS)

    def body():
        # ---------------- per-iteration activations ------------------------
        visS = actp.tile([P, KT1, DIM], dt.float8e4, name="visS", tag="visS")
        bevS = actp.tile([P, KTB, HW], dt.float8e4, name="bevS", tag="bevS")
        h1T = actp.tile([P, KTD, NVS], dt.bfloat16, name="h1T", tag="h1T")
        h2T = actp.tile([P, KTD, HW], dt.bfloat16, name="h2T", tag="h2T")
        vhT = actp.tile([P, KTD, NVS], dt.bfloat16, name="vhT", tag="vhT")
        qT = actp.tile([P, KTD, NVS], dt.bfloat16, name="qT", tag="qT")
        bhT = actp.tile([P, KTD, HW], dt.float8e4, name="bhT", tag="bhT")
        kT = actp.tile([P, KTD, HW], dt.bfloat16, name="kT", tag="kT")
        v_ext = actp.tile([P, HWT, NH, 66], dt.float8e4, name="v_ext", tag="v_ext")
        aoT = actp.tile([HD, NH, NVS], dt.float8e4, name="aoT", tag="aoT")
        xT = actp.tile([P, KTD, NVS], dt.bfloat16, name="xT", tag="xT")
        xsq = actp.tile([P, KTD, NVS], dt.bfloat16, name="xsq", tag="xsq")
        # single-partition scratch rows: [0:512] mean, [512:1024] rstd
        rows = actp.tile([1, 2 * NVS], dt.bfloat16, name="rows", tag="rows")
        rtmp = actp.tile([1, 2 * NVS], dt.bfloat16, name="rtmp", tag="rtmp")
        fusedT = actp.tile([P, KTD, NVS], dt.bfloat16, name="fusedT", tag="fusedT")
        h3F = actp.tile([P, KTD, NVS], dt.float8e4, name="h3F", tag="h3F")

        nc.vector.memset(v_ext[:, :, :, 64:66], 1.0)

        # ====== phase A: vis MLP L1 and BEV L1, fp8 DoubleRow, interleaved ==
        pmA = [wtile(psW), wtile(psW)]

        def mlp1_step(s):
            if s % 2 == 0:
                c = s // 2  # 7 chunks of 4 k-tiles
                nc.sync.dma_start(
                    visS[:, 4 * c:4 * c + 4, :],
                    d["visF"][4 * c:4 * c + 4].rearrange("a p n -> p a n"),
                )
            for mt in range(KTD):
                nc.tensor.matmul(
                    pmA[mt // 2][:, (mt % 2) * 512:(mt % 2 + 1) * 512],
                    w1aS[:, 2 * s:2 * s + 2, mt * P:(mt + 1) * P],
                    visS[:, 2 * s:2 * s + 2, :],
                    start=(s == 0), stop=(s == 13), perf_mode=DR,
                )
            if s == 13:
                for m in range(2):
                    nc.scalar.activation(h1T[:, 2 * m:2 * m + 2, :], pmA[m][:], AF.Gelu)

        pmB = [None]

        def bev_step(j):
            n, ks = j // 8, j % 8
            if ks == 0:
                pmB[0] = [wtile(psX), wtile(psX)]
            if n == 0 and ks % 2 == 0:
                c = ks // 2  # 4 chunks of 4 k-tiles
                nc.sync.dma_start(
                    bevS[:, 4 * c:4 * c + 4, :],
                    d["bevF"][4 * c:4 * c + 4].rearrange("a p n -> p a n"),
                )
            for mt in range(KTD):
                nc.tensor.matmul(
                    pmB[0][mt // 2][:, (mt % 2) * 512:(mt % 2 + 1) * 512],
                    w2aS[:, 2 * ks:2 * ks + 2, mt * P:(mt + 1) * P],
                    bevS[:, 2 * ks:2 * ks + 2, n * 512:(n + 1) * 512],
                    start=(ks == 0), stop=(ks == 7), perf_mode=DR,
                )
            if ks == 7:
                for m in range(2):
                    nc.scalar.activation(
                        h2T[:, 2 * m:2 * m + 2, n * 512:(n + 1) * 512],
                        pmB[0][m][:].rearrange("p (a n) -> p a n", a=2), AF.Gelu)

        m1_done, bev_done = 0, 0
        for step in range(14 + 16):
            run_m1 = (step < 2 or step % 2 == 0) and m1_done < 14
            if run_m1 or bev_done >= 16:
                mlp1_step(m1_done)
                m1_done += 1
            else:
                bev_step(bev_done)
                bev_done += 1

        if PHASES < 2:
            return
        # ====== phase B: projections (wide psum tiles, paired drains) =======
        def proj_pair(pool, out_ap_fn, stat_fn, moving_fn, drain):
            # two 512-wide outputs accumulated into one wide tile, one drain
            pm = wtile(pool)
            for half in range(2):
                for kt in range(KTD):
                    nc.tensor.matmul(
                        pm[:, half * 512:(half + 1) * 512],
                        stat_fn(half, kt), moving_fn(half, kt),
                        start=(kt == 0), stop=(kt == KTD - 1),
                    )
            drain(out_ap_fn(), pm[:].rearrange("p (a n) -> p a n", a=2))

        # vhT = W1b h1T
        for mp in range(2):
            proj_pair(
                psW,
                lambda mp=mp: vhT[:, 2 * mp:2 * mp + 2, :],
                lambda half, kt, mp=mp: wsm[:, W1B, kt, (2 * mp + half) * P:(2 * mp + half + 1) * P],
                lambda half, kt: h1T[:, kt, :],
                nc.vector.tensor_copy,
            )

        # bhT = W2b h2T  (DVE drains so ACT can pull the exp table load and
        # the first score exps forward into phase B)
        def bh_half(n):
            for mp in range(2):
                proj_pair(
                    psX,
                    lambda mp=mp, n=n: bhT[:, 2 * mp:2 * mp + 2, n * 512:(n + 1) * 512],
                    lambda half, kt, mp=mp: wsm[:, W2B, kt, (2 * mp + half) * P:(2 * mp + half + 1) * P],
                    lambda half, kt, n=n: h2T[:, kt, n * 512:(n + 1) * 512],
                    nc.vector.tensor_copy,
                )

        bh_half(0)

        # qT = Wq vhT
        for mp in range(2):
            proj_pair(
                psW,
                lambda mp=mp: qT[:, 2 * mp:2 * mp + 2, :],
                lambda half, kt, mp=mp: wsm[:, WQ, kt, (2 * mp + half) * P:(2 * mp + half + 1) * P],
                lambda half, kt: vhT[:, kt, :],
                nc.vector.tensor_copy,
            )

        bh_half(1)

        # v (token-major, fp8, ones rider col 64) = bhT^T Wv -- before kT so
        # the ao pipeline can start as soon as each head's exp lands
        for q in range(4):
            pm = wtile(psX)
            for half in range(2):
                hw = 2 * q + half
                for kp in range(KTD // 2):
                    nc.tensor.matmul(
                        pm[:, half * 512:(half + 1) * 512],
                        bhT[:, 2 * kp:2 * kp + 2, hw * P:(hw + 1) * P],
                        wkv8[:, 1, 2 * kp:2 * kp + 2, :],
                        start=(kp == 0), stop=(kp == KTD // 2 - 1), perf_mode=DR,
                    )
            nc.vector.tensor_copy(
                v_ext[:, 2 * q:2 * q + 2, :, 0:64],
                pm[:].rearrange("p (a h e) -> p a h e", a=2, h=NH),
            )

        # kT = Wk bhT (fp8 DoubleRow)
        def k_half(n):
            for mp in range(2):
                pm = wtile(psW)
                for half in range(2):
                    mt = 2 * mp + half
                    for kp in range(KTD // 2):
                        nc.tensor.matmul(
                            pm[:, half * 512:(half + 1) * 512],
                            wkv8[:, 0, 2 * kp:2 * kp + 2, mt * P:(mt + 1) * P],
                            bhT[:, 2 * kp:2 * kp + 2, n * 512:(n + 1) * 512],
                            start=(kp == 0), stop=(kp == KTD // 2 - 1), perf_mode=DR,
                        )
                nc.vector.tensor_copy(
                    kT[:, 2 * mp:2 * mp + 2, n * 512:(n + 1) * 512],
                    pm[:].rearrange("p (a n) -> p a n", a=2))

        k_half(0)
        k_half(1)

        if PHASES < 3:
            return
        # ====== phase C: attention, head PAIRS ==============================
        # pair p: head 2p on PE rows 0:63, head 2p+1 on rows 64:127.
        exp_tiles = {}
        ao_tiles = {}

        def scores_pair(p):
            # expP cols 0:512 = even head, 512:1024 = odd head, per k-tile
            expP = expp.tile([P, HWT, 1024], dt.float8e4, name="expP", tag="expP")
            for kt in range(HWT):
                pm = wtile(psW)
                for par in range(2):  # even head rows 0:64, odd head rows 64:128
                    hp = par * HD
                    nc.tensor.matmul(
                        pm[:, par * 512:(par + 1) * 512],
                        kT[hp:hp + HD, p, kt * P:(kt + 1) * P],
                        qT[hp:hp + HD, p, :],
                        start=True, stop=True,
                    )
                nc.scalar.activation(expP[:, kt, :], pm[:], AF.Exp, scale=0.125)
            exp_tiles[p] = expP

        def ao_pair(p):
            expP = exp_tiles.pop(p)
            pmx = wtile(psX)
            for k in range(HWT // 2):
                for par in range(2):
                    nc.tensor.matmul(
                        pmx[0:65, par * 512:(par + 1) * 512],
                        v_ext[:, 2 * k:2 * k + 2, 2 * p + par, 0:65],
                        expP[:, 2 * k:2 * k + 2, par * 512:(par + 1) * 512],
                        start=(k == 0), stop=(k == HWT // 2 - 1), perf_mode=DR,
                    )
            ao_tiles[p] = pmx

        def tail_pair(p):
            pmx = ao_tiles.pop(p)
            rc = rcp.tile([65, 1024], dt.bfloat16, name="rc", tag="rc")
            with nc.allow_low_precision(reason="softmax denom ~1e3, bf16 recip is plenty"):
                nc.vector.reciprocal(rc[64:65, :], pmx[64:65, :])
            # broadcast the reciprocal row to 64 partitions with an
            # SBUF->SBUF DMA (0-stride partition source): keeps the tail off
            # the PE and out of the PSUM pools so it can't stall the
            # scores->exp rotation.
            bc = bcp.tile([HD, 2, NVS], dt.bfloat16, name="bc", tag="bc")
            nc.sync.dma_start(
                bc[:], rc[64:65, :].unsqueeze(1).broadcast_to((1, HD, 1024)))
            with nc.allow_low_precision(reason="attention out in fp8, tol 2e-2"):
                nc.vector.tensor_mul(
                    aoT[:, 2 * p:2 * p + 2, :],
                    pmx[0:HD, :].rearrange("p (a n) -> p a n", a=2),
                    bc[:])

        KC = int(os.environ.get("KC", "3"))  # debug: 1=scores only, 2=+ao, 3=full
        if KC == 1:
            for p in range(NPAIR):
                scores_pair(p)
                exp_tiles.pop(p)
        elif KC == 2:
            scores_pair(0)
            scores_pair(1)
            ao_pair(0)
            scores_pair(2)
            ao_pair(1)
            scores_pair(3)
            ao_pair(2)
            ao_pair(3)
            for p in range(NPAIR):
                ao_tiles.pop(p)
        else:
            scores_pair(0)
            scores_pair(1)
            ao_pair(0)
            scores_pair(2)
            tail_pair(0)
            ao_pair(1)
            scores_pair(3)
            tail_pair(1)
            ao_pair(2)
            tail_pair(2)
            ao_pair(3)
            tail_pair(3)

        if PHASES < 4:
            return
        # ====== phase D: Wo (feature-major), x = vh + ao Wo^T, LayerNorm ====
        pwo = [wtile(psX), wtile(psX)]
        for dtile in range(KTD):
            for hp2 in range(NH // 2):
                nc.tensor.matmul(
                    pwo[dtile // 2][:, (dtile % 2) * 512:(dtile % 2 + 1) * 512],
                    woF[:, 2 * hp2:2 * hp2 + 2, dtile, :], aoT[:, 2 * hp2:2 * hp2 + 2, :],
                    start=(hp2 == 0), stop=(hp2 == NH // 2 - 1), perf_mode=DR,
                )
        for m in range(2):
            nc.vector.tensor_add(
                xT[:, 2 * m:2 * m + 2, :],
                pwo[m][:].rearrange("p (a n) -> p a n", a=2),
                vhT[:, 2 * m:2 * m + 2, :])
            nc.scalar.activation(xsq[:, 2 * m:2 * m + 2, :], xT[:, 2 * m:2 * m + 2, :], AF.Square)

        # stats (ones are 1/DIM): S1 = mean (bank 0), S2 = E[x^2] (bank 1)
        pst = wtile(psW)
        for kt in range(KTD):
            nc.tensor.matmul(pst[0:1, 0:512], ones_c[:, 0:1], xT[:, kt, :],
                             start=(kt == 0), stop=(kt == KTD - 1))
        for kt in range(KTD):
            nc.tensor.matmul(pst[0:1, 512:1024], ones_c[:, 1:2], xsq[:, kt, :],
                             start=(kt == 0), stop=(kt == KTD - 1))

        # row math, all on partition 0: mean in rows[0:512], rstd in rows[512:]
        with nc.allow_low_precision(reason="LN stats in bf16, tol 2e-2"):
            nc.vector.tensor_copy(rows[0:1, 0:NVS], pst[0:1, 0:512])
            nc.vector.tensor_mul(rtmp[0:1, NVS:], rows[0:1, 0:NVS], rows[0:1, 0:NVS])
            # var = E[x^2]*1 - mean^2, fused psum read + subtract
            nc.vector.scalar_tensor_tensor(
                rtmp[0:1, 0:NVS], pst[0:1, 512:1024], 1.0, rtmp[0:1, NVS:],
                op0=mybir.AluOpType.mult, op1=mybir.AluOpType.subtract)
        nc.scalar.activation(rtmp[0:1, NVS:], rtmp[0:1, 0:NVS], AF.Ln, bias=eps_t[:])
        nc.scalar.activation(rows[0:1, NVS:], rtmp[0:1, NVS:], AF.Exp, scale=-0.5)
        # dummy 1-elem gelu: pulls the ~2.7us natural_log_exp->gelu table
        # switch into ACT's idle stretch here instead of phase E's critical
        # path (no further exp/ln uses this iteration).
        nc.scalar.activation(rtmp[0:1, 2 * NVS - 1:], rtmp[0:1, 2 * NVS - 1:], AF.Gelu)

        # broadcast mean and rstd to all 128 rows, both banks of each tile
        # (psX so phase E's W3b psW rotation isn't gated on the ft reads)
        pmb = wtile(psX)   # mean x2 banks
        pmr = wtile(psX)   # rstd x2 banks
        for bank in range(2):
            nc.tensor.matmul(pmb[:, bank * 512:(bank + 1) * 512],
                             ones_r[0:1, :], rows[0:1, 0:NVS],
                             start=True, stop=True)
            nc.tensor.matmul(pmr[:, bank * 512:(bank + 1) * 512],
                             ones_r[0:1, :], rows[0:1, NVS:],
                             start=True, stop=True)

        for half in range(2):
            ft = ftp.tile([P, 2, NVS], dt.bfloat16, name="ft", tag="ft")
            nc.vector.tensor_sub(
                ft[:], xT[:, 2 * half:2 * half + 2, :],
                pmb[:].rearrange("p (a n) -> p a n", a=2))
            nc.vector.tensor_mul(
                fusedT[:, 2 * half:2 * half + 2, :], ft[:],
                pmr[:].rearrange("p (a n) -> p a n", a=2))

        if PHASES < 5:
            return
        # ====== phase E: output MLP, fp8 DoubleRow for W3b ==================
        for mp in range(2):
            pm = wtile(psX)
            for half in range(2):
                mt = 2 * mp + half
                for kt in range(KTD):
                    nc.tensor.matmul(
                        pm[:, half * 512:(half + 1) * 512],
                        wsm[:, W3A, kt, mt * P:(mt + 1) * P], fusedT[:, kt, :],
                        start=(kt == 0), stop=(kt == KTD - 1),
                    )
            nc.scalar.activation(h3F[:, 2 * mp:2 * mp + 2, :], pm[:].rearrange("p (a n) -> p a n", a=2),
                                 AF.Gelu)

        for n in range(NO3):
            dstage = outp.tile([P, MT, 512], dt.bfloat16, name="dstage", tag="dstage")
            for mp in range(2):
                pm = wtile(psW if (n + mp) % 2 == 0 else psX)
                for half in range(2):
                    mt = 2 * mp + half
                    for kp in range(KTD // 2):
                        nc.tensor.matmul(
                            pm[:, half * 512:(half + 1) * 512],
                            h3F[:, 2 * kp:2 * kp + 2, mt * P:(mt + 1) * P],
                            w3bS[:, 2 * kp:2 * kp + 2, n * 512:(n + 1) * 512],
                            start=(kp == 0), stop=(kp == KTD // 2 - 1), perf_mode=DR,
                        )
                nc.vector.tensor_copy(
                    dstage[:, 2 * mp:2 * mp + 2, :],
                    pm[:].rearrange("p (a n) -> p a n", a=2))
            nc.sync.dma_start(
                d["delta"].rearrange("(m p) n -> p m n", p=P)[:, :, n * 512:(n + 1) * 512],
                dstage[:],
            )

    if reps > 1:
        with tc.For_i(0, reps, 1):
            body()
    else:
        body()

    for p in (psX, psW, outp, ftp, bcp, rcp, expp, actp, const):
        p.release()


@functools.lru_cache(maxsize=4)
def _build(reps):
    import concourse.tile as tile
    from concourse import bacc, mybir

    dt = mybir.dt
    nc = bacc.Bacc("TRN2", target_bir_lowering=False, debug=False)
    d = {
        "visF": nc.dram_tensor("visF", [KT1, P, DIM], dt.float8e4, kind="ExternalInput").ap(),
        "bevF": nc.dram_tensor("bevF", [KTB, P, HW], dt.float8e4, kind="ExternalInput").ap(),
        "w1aF": nc.dram_tensor("w1aF", [KT1, P, DIM], dt.float8e4, kind="ExternalInput").ap(),
        "w2aF": nc.dram_tensor("w2aF", [KTB, P, DIM], dt.float8e4, kind="ExternalInput").ap(),
        "w3bF": nc.dram_tensor("w3bF", [KTD, P, HID], dt.float8e4, kind="ExternalInput").ap(),
        "wsmF": nc.dram_tensor("wsmF", [4, KTD, P, DIM], dt.bfloat16, kind="ExternalInput").ap(),
        "wkv8": nc.dram_tensor("wkv8", [2, KTD, P, DIM], dt.float8e4, kind="ExternalInput").ap(),
        "woF": nc.dram_tensor("woF", [HD, NH, KTD, P], dt.float8e4, kind="ExternalInput").ap(),
        "delta": nc.dram_tensor("delta", [NVS, HID], dt.bfloat16, kind="ExternalOutput").ap(),
    }
    with tile.TileContext(nc) as tc:
        _emit(nc, tc, d, reps)
    nc.compile()
    return nc


def _host_prep(inputs):
    hs = np.asarray(inputs["hidden_states"], dtype=np.float32)
    bev = np.asarray(inputs["bev_feat"], dtype=np.float32)
    vis_idx = np.asarray(inputs["vis_idx"])

    def ktile(mat_t, kt):
        # [K, N] -> [kt, 128, N]
        return np.ascontiguousarray(mat_t).reshape(kt, P, -1)

    w1aF = ktile(np.asarray(inputs["w1a"], np.float32).T, KT1).astype(f8)
    w2aF = ktile(np.asarray(inputs["w2a"], np.float32).T, KTB).astype(f8)
    w3bF = ktile(np.asarray(inputs["w3b"], np.float32).T, KTD).astype(f8)
    wsmF = np.stack(
        [
            ktile(np.asarray(inputs[k], np.float32).T, KTD)
            for k in ("w1b", "w2b", "wq", "w3a")
        ]
    ).astype(bf16)
    wkv8 = np.stack(
        [
            ktile(np.asarray(inputs[k], np.float32).T, KTD)
            for k in ("wk", "wv")
        ]
    ).astype(f8)
    # woF[p, h, dt, m] = Wo[dt*128+m, h*64+p]
    wo = np.asarray(inputs["wo"], np.float32)         # [out, in]
    woF = np.ascontiguousarray(
        wo.T.reshape(NH, HD, KTD, P).transpose(1, 0, 2, 3)
    ).astype(f8)

    vis_by_b = [hs[b][vis_idx[b]] for b in range(B)]  # [NV, HID] f32 each
    in_maps = []
    for c in range(NCORES):
        b, half = c // 2, c % 2
        vis_half = vis_by_b[b][half * NVS:(half + 1) * NVS]
        in_maps.append(
            {
                "visF": ktile(vis_half.T, KT1).astype(f8),
                "bevF": ktile(bev[b].reshape(BEV, HW), KTB).astype(f8),
                "w1aF": w1aF,
                "w2aF": w2aF,
                "w3bF": w3bF,
                "wsmF": wsmF,
                "wkv8": wkv8,
                "woF": woF,
            }
        )
    return hs, vis_idx, vis_by_b, in_maps


def kernel(**inputs):
    from concourse import bass_utils

    nc = _build(REPS)
    hs, vis_idx, vis_by_b, in_maps = _host_prep(inputs)
    res = bass_utils.run_bass_kernel_spmd(nc, in_maps, core_ids=list(range(NCORES)))

    out = hs.copy()
    for c in range(NCORES):
        b, half = c // 2, c % 2
        delta = res.results[c]["delta"].astype(np.float32)
        enh = vis_by_b[b][half * NVS:(half + 1) * NVS] + delta
        out[b][vis_idx[b][half * NVS:(half + 1) * NVS]] = enh
    return out


# revision 26
# speedup vs baseline: 1.0821x; 1.0821x over previous
"""Trainium2 Bass kernel for BEVLayerInjector (8-core SPMD), v3.

Sharding: data-parallel over batch B=4 x 2-way split of the NV=1024 gathered
vision tokens -> 8 shards, one per NeuronCore. The gather (hidden_states ->
vis) and the final scatter + residual add run on the host; the device computes
delta = MLP3(LN(vh + CrossAttn(vh, bh) @ Wo^T)) per shard.

v3 changes vs v2 (which measured ~184us/iter, phase C alone 85us):
  - phase C processes heads in PAIRS: even head on PE rows 0:63, odd head on
    rows 64:127.  Score matmuls for the two heads are emitted interleaved so
    the 64-row subarray groups run concurrently (~2x PE throughput on the
    K=64 score matmuls), and each k-tile's pair of score blocks is drained by
    ONE ACT exp call [128,1024] (ACT is the phase-C bottleneck engine).
  - softmax normalization: per pair ONE DVE reciprocal [1,1024] on the
    ones-rider denominator rows, gpsimd partition_broadcast to 64 rows (was:
    PE broadcast matmul + DVE copy), and ONE DVE multiply for both heads.
    Removes 12 PE matmuls and shortens the per-head cross-engine chain.
  - ao accumulates per pair in one 2-bank psX tile (even head bank a, odd
    head bank b) - no extra PSUM pools.
  - LayerNorm: stats ones are pre-scaled by 1/DIM so mean/E[x^2] come out of
    PSUM directly; var fused into one scalar_tensor_tensor; mean/rstd
    broadcast matmuls write both banks so the final normalize is 4 batched
    DVE ops instead of 8.
  - dead wk/wv copies dropped from the packed bf16 weight bundle (wsm now
    holds only w1b, w2b, wq, w3a).

Layout: "feature-major" tensors keep the contraction dim on SBUF partitions
([feature, token]); weights are shipped pre-transposed as [in, out] and
k-tiled as [ktile, 128, out] so every DMA is contiguous.
"""

import functools
import os
import sys

sys.path.insert(0, "/opt/trn_rl_repo")

import numpy as np
import ml_dtypes

B, S, HID = 4, 4096, 3584
BEV, DIM, NH = 2048, 512, 8
NV, HW, HD = 1024, 1024, 64
EPS = 1e-5
P = 128
NCORES = 8
NVS = NV // 2              # tokens per core (512)
KT1 = HID // P             # 28 k-tiles for HID
KTD = DIM // P             # 4 k-tiles for DIM
KTB = BEV // P             # 16 k-tiles for BEV
MT = NVS // P              # 4 token m-tiles
HWT = HW // P              # 8 HW k-tiles
NO3 = HID // 512           # 7 output column chunks
NPAIR = NH // 2            # 4 head pairs

# index of each 512x512 weight (pre-transposed to [in, out]) in the packed wsm
W1B, W2B, WQ, W3A = range(4)
# fp8 copies for the DoubleRow projections
W2B8, WQ8, W3A8 = range(3)

REPS = 1  # how many times the body is emitted (timing builds use >1)
PHASES = int(os.environ.get("KPHASES", "5"))  # debug: truncate body after N phases
# fp8 DoubleRow knobs for the DIM->DIM projections (1 = fp8-DR, 0 = bf16)
K8BH = int(os.environ.get("K8BH", "1"))
K8Q = int(os.environ.get("K8Q", "1"))
K8MLP3 = int(os.environ.get("K8MLP3", "1"))

bf16 = ml_dtypes.bfloat16
f8 = ml_dtypes.float8_e4m3


def _emit(nc, tc, d, reps):
    from concourse import mybir

    dt = mybir.dt
    AF = mybir.ActivationFunctionType
    DR = mybir.MatmulPerfMode.DoubleRow

    const = tc.alloc_tile_pool(name="const", bufs=1)
    actp = tc.alloc_tile_pool(name="actp", bufs=1)
    expp = tc.alloc_tile_pool(name="expp", bufs=2)
    rcp = tc.alloc_tile_pool(name="rcp", bufs=2)
    bcp = tc.alloc_tile_pool(name="bcp", bufs=2)
    ftp = tc.alloc_tile_pool(name="ftp", bufs=2)
    outp = tc.alloc_tile_pool(name="outp", bufs=2)
    # two rotating pools of 2-bank-wide psum tiles: 2x2x2KB each = all 8 banks
    psW = tc.alloc_tile_pool(name="psW", bufs=2, space="PSUM")
    psX = tc.alloc_tile_pool(name="psX", bufs=2, space="PSUM")

    def wtile(pool):
        tag = "w" if pool is psW else "x"
        return pool.tile([P, 1024], dt.float32, name=tag, tag=tag)

    # ---------------- resident weights (loaded once, before the loop) -------
    w1aS = const.tile([P, KT1, DIM], dt.float8e4, name="w1aS", tag="w1aS")
    w2aS = const.tile([P, KTB, DIM], dt.float8e4, name="w2aS", tag="w2aS")
    w3bS = const.tile([P, KTD, HID], dt.float8e4, name="w3bS", tag="w3bS")
    wsm = const.tile([P, 4, KTD, DIM], dt.bfloat16, name="wsm", tag="wsm")
    wsm8 = const.tile([P, 3, KTD, DIM], dt.float8e4, name="wsm8", tag="wsm8")
    wkv8 = const.tile([P, 2, KTD, DIM], dt.float8e4, name="wkv8", tag="wkv8")
    woF = const.tile([HD, NH, KTD, P], dt.float8e4, name="woF", tag="woF")
    for c in range(2):
        nc.sync.dma_start(
            w1aS[:, c * 14:(c + 1) * 14, :],
            d["w1aF"][c * 14:(c + 1) * 14].rearrange("a p n -> p a n"),
        )
    nc.sync.dma_start(w2aS[:], d["w2aF"].rearrange("a p n -> p a n"))
    nc.sync.dma_start(w3bS[:], d["w3bF"].rearrange("a p n -> p a n"))
    nc.sync.dma_start(wsm[:], d["wsmF"].rearrange("w a p n -> p w a n"))
    nc.sync.dma_start(wsm8[:], d["wsm8F"].rearrange("w a p n -> p w a n"))
    nc.sync.dma_start(wkv8[:], d["wkv8"].rearrange("w a p n -> p w a n"))
    nc.sync.dma_start(woF[:], d["woF"])

    ones_c = const.tile([P, 2], dt.bfloat16, name="ones_c", tag="ones_c")
    nc.vector.memset(ones_c[:], 1.0 / DIM)   # stats matmuls emit mean directly
    ones_r = const.tile([P, P], dt.bfloat16, name="ones_r", tag="ones_r")
    nc.vector.memset(ones_r[:], 1.0)
    eps_t = const.tile([1, 1], dt.float32, name="eps", tag="eps")
    nc.vector.memset(eps_t[:], EPS)

    def body():
        # ---------------- per-iteration activations ------------------------
        visS = actp.tile([P, KT1, DIM], dt.float8e4, name="visS", tag="visS")
        bevS = actp.tile([P, KTB, HW], dt.float8e4, name="bevS", tag="bevS")
        h1T = actp.tile([P, KTD, NVS], dt.bfloat16, name="h1T", tag="h1T")
        h2T = actp.tile([P, KTD, HW], dt.float8e4 if K8BH else dt.bfloat16,
                        name="h2T", tag="h2T")
        vhT = actp.tile([P, KTD, NVS], dt.bfloat16, name="vhT", tag="vhT")
        vhT8 = actp.tile([P, KTD, NVS], dt.float8e4, name="vhT8", tag="vhT8")
        qT = actp.tile([P, KTD, NVS], dt.bfloat16, name="qT", tag="qT")
        bhT = actp.tile([P, KTD, HW], dt.float8e4, name="bhT", tag="bhT")
        kT = actp.tile([P, KTD, HW], dt.bfloat16, name="kT", tag="kT")
        v_ext = actp.tile([P, HWT, NH, 66], dt.float8e4, name="v_ext", tag="v_ext")
        aoT = actp.tile([HD, NH, NVS], dt.float8e4, name="aoT", tag="aoT")
        xT = actp.tile([P, KTD, NVS], dt.bfloat16, name="xT", tag="xT")
        xsq = actp.tile([P, KTD, NVS], dt.bfloat16, name="xsq", tag="xsq")
        # single-partition scratch rows: [0:512] mean, [512:1024] rstd
        rows = actp.tile([1, 2 * NVS], dt.bfloat16, name="rows", tag="rows")
        rtmp = actp.tile([1, 2 * NVS], dt.bfloat16, name="rtmp", tag="rtmp")
        fusedT = actp.tile([P, KTD, NVS], dt.float8e4 if K8MLP3 else dt.bfloat16,
                           name="fusedT", tag="fusedT")
        h3F = actp.tile([P, KTD, NVS], dt.float8e4, name="h3F", tag="h3F")

        nc.vector.memset(v_ext[:, :, :, 64:66], 1.0)

        # ====== phase A: vis MLP L1 and BEV L1, fp8 DoubleRow, interleaved ==
        pmA = [wtile(psW), wtile(psW)]

        def mlp1_step(s):
            if s % 2 == 0:
                c = s // 2  # 7 chunks of 4 k-tiles
                nc.sync.dma_start(
                    visS[:, 4 * c:4 * c + 4, :],
                    d["visF"][4 * c:4 * c + 4].rearrange("a p n -> p a n"),
                )
            for mt in range(KTD):
                nc.tensor.matmul(
                    pmA[mt // 2][:, (mt % 2) * 512:(mt % 2 + 1) * 512],
                    w1aS[:, 2 * s:2 * s + 2, mt * P:(mt + 1) * P],
                    visS[:, 2 * s:2 * s + 2, :],
                    start=(s == 0), stop=(s == 13), perf_mode=DR,
                )
            if s == 13:
                for m in range(2):
                    nc.scalar.activation(h1T[:, 2 * m:2 * m + 2, :], pmA[m][:], AF.Gelu)

        pmB = [None]

        def bev_step(j):
            n, ks = j // 8, j % 8
            if ks == 0:
                pmB[0] = [wtile(psX), wtile(psX)]
            if n == 0 and ks % 2 == 0:
                c = ks // 2  # 4 chunks of 4 k-tiles
                nc.sync.dma_start(
                    bevS[:, 4 * c:4 * c + 4, :],
                    d["bevF"][4 * c:4 * c + 4].rearrange("a p n -> p a n"),
                )
            for mt in range(KTD):
                nc.tensor.matmul(
                    pmB[0][mt // 2][:, (mt % 2) * 512:(mt % 2 + 1) * 512],
                    w2aS[:, 2 * ks:2 * ks + 2, mt * P:(mt + 1) * P],
                    bevS[:, 2 * ks:2 * ks + 2, n * 512:(n + 1) * 512],
                    start=(ks == 0), stop=(ks == 7), perf_mode=DR,
                )
            if ks == 7:
                for m in range(2):
                    nc.scalar.activation(
                        h2T[:, 2 * m:2 * m + 2, n * 512:(n + 1) * 512],
                        pmB[0][m][:].rearrange("p (a n) -> p a n", a=2), AF.Gelu)

        m1_done, bev_done = 0, 0
        for step in range(14 + 16):
            run_m1 = (step < 2 or step % 2 == 0) and m1_done < 14
            if run_m1 or bev_done >= 16:
                mlp1_step(m1_done)
                m1_done += 1
            else:
                bev_step(bev_done)
                bev_done += 1

        if PHASES < 2:
            return
        # ====== phase B: projections (wide psum tiles, paired drains) =======
        def proj_pair(pool, out_ap_fn, stat_fn, moving_fn, drain):
            # two 512-wide outputs accumulated into one wide tile, one drain
            pm = wtile(pool)
            for half in range(2):
                for kt in range(KTD):
                    nc.tensor.matmul(
                        pm[:, half * 512:(half + 1) * 512],
                        stat_fn(half, kt), moving_fn(half, kt),
                        start=(kt == 0), stop=(kt == KTD - 1),
                    )
            drain(out_ap_fn(), pm[:].rearrange("p (a n) -> p a n", a=2))

        # vhT = W1b h1T (dual drains: bf16 for the residual, fp8 for the q-DR)
        for mp in range(2):
            pm = wtile(psW)
            for half in range(2):
                for kt in range(KTD):
                    nc.tensor.matmul(
                        pm[:, half * 512:(half + 1) * 512],
                        wsm[:, W1B, kt, (2 * mp + half) * P:(2 * mp + half + 1) * P],
                        h1T[:, kt, :],
                        start=(kt == 0), stop=(kt == KTD - 1),
                    )
            nc.vector.tensor_copy(vhT[:, 2 * mp:2 * mp + 2, :],
                                  pm[:].rearrange("p (a n) -> p a n", a=2))
            if K8Q:
                with nc.allow_low_precision(reason="fp8 copy feeds q projection only"):
                    nc.gpsimd.tensor_copy(vhT8[:, 2 * mp:2 * mp + 2, :],
                                          vhT[:, 2 * mp:2 * mp + 2, :])

        # bhT = W2b h2T  (DVE drains so ACT can pull the exp table load and
        # the first score exps forward into phase B)
        def bh_half(n):
            for mp in range(2):
                if K8BH:
                    pm = wtile(psX)
                    for half in range(2):
                        mt = 2 * mp + half
                        for kp in range(KTD // 2):
                            nc.tensor.matmul(
                                pm[:, half * 512:(half + 1) * 512],
                                wsm8[:, W2B8, 2 * kp:2 * kp + 2, mt * P:(mt + 1) * P],
                                h2T[:, 2 * kp:2 * kp + 2, n * 512:(n + 1) * 512],
                                start=(kp == 0), stop=(kp == KTD // 2 - 1), perf_mode=DR,
                            )
                    nc.scalar.copy(
                        bhT[:, 2 * mp:2 * mp + 2, n * 512:(n + 1) * 512],
                        pm[:].rearrange("p (a n) -> p a n", a=2))
                else:
                    proj_pair(
                        psX,
                        lambda mp=mp, n=n: bhT[:, 2 * mp:2 * mp + 2, n * 512:(n + 1) * 512],
                        lambda half, kt, mp=mp: wsm[:, W2B, kt, (2 * mp + half) * P:(2 * mp + half + 1) * P],
                        lambda half, kt, n=n: h2T[:, kt, n * 512:(n + 1) * 512],
                        nc.scalar.copy,
                    )

        bh_half(0)

        # qT = Wq vhT
        for mp in range(2):
            if K8Q:
                pm = wtile(psW)
                for half in range(2):
                    mt = 2 * mp + half
                    for kp in range(KTD // 2):
                        nc.tensor.matmul(
                            pm[:, half * 512:(half + 1) * 512],
                            wsm8[:, WQ8, 2 * kp:2 * kp + 2, mt * P:(mt + 1) * P],
                            vhT8[:, 2 * kp:2 * kp + 2, :],
                            start=(kp == 0), stop=(kp == KTD // 2 - 1), perf_mode=DR,
                        )
                nc.vector.tensor_copy(qT[:, 2 * mp:2 * mp + 2, :],
                                      pm[:].rearrange("p (a n) -> p a n", a=2))
            else:
                proj_pair(
                    psW,
                    lambda mp=mp: qT[:, 2 * mp:2 * mp + 2, :],
                    lambda half, kt, mp=mp: wsm[:, WQ, kt, (2 * mp + half) * P:(2 * mp + half + 1) * P],
                    lambda half, kt: vhT[:, kt, :],
                    nc.vector.tensor_copy,
                )

        bh_half(1)

        # v (token-major, fp8, ones rider col 64) = bhT^T Wv -- before kT so
        # the ao pipeline can start as soon as each head's exp lands
        for q in range(4):
            pm = wtile(psX)
            for half in range(2):
                hw = 2 * q + half
                for kp in range(KTD // 2):
                    nc.tensor.matmul(
                        pm[:, half * 512:(half + 1) * 512],
                        bhT[:, 2 * kp:2 * kp + 2, hw * P:(hw + 1) * P],
                        wkv8[:, 1, 2 * kp:2 * kp + 2, :],
                        start=(kp == 0), stop=(kp == KTD // 2 - 1), perf_mode=DR,
                    )
            nc.scalar.copy(
                v_ext[:, 2 * q:2 * q + 2, :, 0:64],
                pm[:].rearrange("p (a h e) -> p a h e", a=2, h=NH),
            )

        # kT = Wk bhT (fp8 DoubleRow)
        def k_half(n):
            for mp in range(2):
                pm = wtile(psW)
                for half in range(2):
                    mt = 2 * mp + half
                    for kp in range(KTD // 2):
                        nc.tensor.matmul(
                            pm[:, half * 512:(half + 1) * 512],
                            wkv8[:, 0, 2 * kp:2 * kp + 2, mt * P:(mt + 1) * P],
                            bhT[:, 2 * kp:2 * kp + 2, n * 512:(n + 1) * 512],
                            start=(kp == 0), stop=(kp == KTD // 2 - 1), perf_mode=DR,
                        )
                nc.vector.tensor_copy(
                    kT[:, 2 * mp:2 * mp + 2, n * 512:(n + 1) * 512],
                    pm[:].rearrange("p (a n) -> p a n", a=2))

        k_half(0)
        k_half(1)

        if PHASES < 3:
            return
        # ====== phase C: attention, head PAIRS ==============================
        # pair p: head 2p on PE rows 0:63, head 2p+1 on rows 64:127.
        exp_tiles = {}
        ao_tiles = {}

        def scores_pair(p):
            # expP cols 0:512 = even head, 512:1024 = odd head, per k-tile
            expP = expp.tile([P, HWT, 1024], dt.float8e4, name="expP", tag="expP")
            for kt in range(HWT):
                pm = wtile(psW)
                for par in range(2):  # even head rows 0:64, odd head rows 64:128
                    hp = par * HD
                    nc.tensor.matmul(
                        pm[:, par * 512:(par + 1) * 512],
                        kT[hp:hp + HD, p, kt * P:(kt + 1) * P],
                        qT[hp:hp + HD, p, :],
                        start=True, stop=True,
                    )
                nc.scalar.activation(expP[:, kt, :], pm[:], AF.Exp, scale=0.125)
            exp_tiles[p] = expP

        def ao_pair(p):
            expP = exp_tiles.pop(p)
            pmx = wtile(psX)
            for k in range(HWT // 2):
                for par in range(2):
                    nc.tensor.matmul(
                        pmx[0:65, par * 512:(par + 1) * 512],
                        v_ext[:, 2 * k:2 * k + 2, 2 * p + par, 0:65],
                        expP[:, 2 * k:2 * k + 2, par * 512:(par + 1) * 512],
                        start=(k == 0), stop=(k == HWT // 2 - 1), perf_mode=DR,
                    )
            ao_tiles[p] = pmx

        def tail_pair(p):
            pmx = ao_tiles.pop(p)
            rc = rcp.tile([65, 1024], dt.bfloat16, name="rc", tag="rc")
            with nc.allow_low_precision(reason="softmax denom ~1e3, bf16 recip is plenty"):
                nc.vector.reciprocal(rc[64:65, :], pmx[64:65, :])
            # broadcast the reciprocal row to 64 partitions with an
            # SBUF->SBUF DMA (0-stride partition source): keeps the tail off
            # the PE and out of the PSUM pools so it can't stall the
            # scores->exp rotation.
            bc = bcp.tile([HD, 2, NVS], dt.bfloat16, name="bc", tag="bc")
            nc.sync.dma_start(
                bc[:], rc[64:65, :].unsqueeze(1).broadcast_to((1, HD, 1024)))
            with nc.allow_low_precision(reason="attention out in fp8, tol 2e-2"):
                nc.vector.tensor_mul(
                    aoT[:, 2 * p:2 * p + 2, :],
                    pmx[0:HD, :].rearrange("p (a n) -> p a n", a=2),
                    bc[:])

        KC = int(os.environ.get("KC", "3"))  # debug: 1=scores only, 2=+ao, 3=full
        if KC == 1:
            for p in range(NPAIR):
                scores_pair(p)
                exp_tiles.pop(p)
        elif KC == 2:
            scores_pair(0)
            scores_pair(1)
            ao_pair(0)
            scores_pair(2)
            ao_pair(1)
            scores_pair(3)
            ao_pair(2)
            ao_pair(3)
            for p in range(NPAIR):
                ao_tiles.pop(p)
        else:
            scores_pair(0)
            scores_pair(1)
            ao_pair(0)
            scores_pair(2)
            tail_pair(0)
            ao_pair(1)
            scores_pair(3)
            tail_pair(1)
            ao_pair(2)
            tail_pair(2)
            ao_pair(3)
            tail_pair(3)

        if PHASES < 4:
            return
        # ====== phase D: Wo (feature-major), x = vh + ao Wo^T, LayerNorm ====
        pwo = [wtile(psX), wtile(psX)]
        for dtile in range(KTD):
            for hp2 in range(NH // 2):
                nc.tensor.matmul(
                    pwo[dtile // 2][:, (dtile % 2) * 512:(dtile % 2 + 1) * 512],
                    woF[:, 2 * hp2:2 * hp2 + 2, dtile, :], aoT[:, 2 * hp2:2 * hp2 + 2, :],
                    start=(hp2 == 0), stop=(hp2 == NH // 2 - 1), perf_mode=DR,
                )
        for m in range(2):
            nc.vector.tensor_add(
                xT[:, 2 * m:2 * m + 2, :],
                pwo[m][:].rearrange("p (a n) -> p a n", a=2),
                vhT[:, 2 * m:2 * m + 2, :])
            nc.scalar.activation(xsq[:, 2 * m:2 * m + 2, :], xT[:, 2 * m:2 * m + 2, :], AF.Square)

        # stats (ones are 1/DIM): S1 = mean (bank 0), S2 = E[x^2] (bank 1)
        pst = wtile(psW)
        for kt in range(KTD):
            nc.tensor.matmul(pst[0:1, 0:512], ones_c[:, 0:1], xT[:, kt, :],
                             start=(kt == 0), stop=(kt == KTD - 1))
        for kt in range(KTD):
            nc.tensor.matmul(pst[0:1, 512:1024], ones_c[:, 1:2], xsq[:, kt, :],
                             start=(kt == 0), stop=(kt == KTD - 1))

        # row math, all on partition 0: mean in rows[0:512], rstd in rows[512:]
        with nc.allow_low_precision(reason="LN stats in bf16, tol 2e-2"):
            nc.vector.tensor_copy(rows[0:1, 0:NVS], pst[0:1, 0:512])
            nc.vector.tensor_mul(rtmp[0:1, NVS:], rows[0:1, 0:NVS], rows[0:1, 0:NVS])
            # var = E[x^2]*1 - mean^2, fused psum read + subtract
            nc.vector.scalar_tensor_tensor(
                rtmp[0:1, 0:NVS], pst[0:1, 512:1024], 1.0, rtmp[0:1, NVS:],
                op0=mybir.AluOpType.mult, op1=mybir.AluOpType.subtract)
        nc.scalar.activation(rtmp[0:1, NVS:], rtmp[0:1, 0:NVS], AF.Ln, bias=eps_t[:])
        nc.scalar.activation(rows[0:1, NVS:], rtmp[0:1, NVS:], AF.Exp, scale=-0.5)
        # dummy 1-elem gelu: pulls the ~2.7us natural_log_exp->gelu table
        # switch into ACT's idle stretch here instead of phase E's critical
        # path (no further exp/ln uses this iteration).
        nc.scalar.activation(rtmp[0:1, 2 * NVS - 1:], rtmp[0:1, 2 * NVS - 1:], AF.Gelu)

        # broadcast mean and rstd to all 128 rows, both banks of each tile
        # (psX so phase E's W3b psW rotation isn't gated on the ft reads)
        pmb = wtile(psX)   # mean x2 banks
        pmr = wtile(psX)   # rstd x2 banks
        for bank in range(2):
            nc.tensor.matmul(pmb[:, bank * 512:(bank + 1) * 512],
                             ones_r[0:1, :], rows[0:1, 0:NVS],
                             start=True, stop=True)
            nc.tensor.matmul(pmr[:, bank * 512:(bank + 1) * 512],
                             ones_r[0:1, :], rows[0:1, NVS:],
                             start=True, stop=True)

        for half in range(2):
            ft = ftp.tile([P, 2, NVS], dt.bfloat16, name="ft", tag="ft")
            nc.vector.tensor_sub(
                ft[:], xT[:, 2 * half:2 * half + 2, :],
                pmb[:].rearrange("p (a n) -> p a n", a=2))
            with nc.allow_low_precision(reason="LN output quantized for fp8 MLP3"):
                nc.vector.tensor_mul(
                    fusedT[:, 2 * half:2 * half + 2, :], ft[:],
                    pmr[:].rearrange("p (a n) -> p a n", a=2))

        if PHASES < 5:
            return
        # ====== phase E: output MLP, fp8 DoubleRow for W3b ==================
        for mp in range(2):
            pm = wtile(psX)
            for half in range(2):
                mt = 2 * mp + half
                if K8MLP3:
                    for kp in range(KTD // 2):
                        nc.tensor.matmul(
                            pm[:, half * 512:(half + 1) * 512],
                            wsm8[:, W3A8, 2 * kp:2 * kp + 2, mt * P:(mt + 1) * P],
                            fusedT[:, 2 * kp:2 * kp + 2, :],
                            start=(kp == 0), stop=(kp == KTD // 2 - 1), perf_mode=DR,
                        )
                else:
                    for kt in range(KTD):
                        nc.tensor.matmul(
                            pm[:, half * 512:(half + 1) * 512],
                            wsm[:, W3A, kt, mt * P:(mt + 1) * P], fusedT[:, kt, :],
                            start=(kt == 0), stop=(kt == KTD - 1),
                        )
            nc.scalar.activation(h3F[:, 2 * mp:2 * mp + 2, :], pm[:].rearrange("p (a n) -> p a n", a=2),
                                 AF.Gelu)

        for n in range(NO3):
            dstage = outp.tile([P, MT, 512], dt.bfloat16, name="dstage", tag="dstage")
            for mp in range(2):
                pm = wtile(psW if (n + mp) % 2 == 0 else psX)
                for half in range(2):
                    mt = 2 * mp + half
                    for kp in range(KTD // 2):
                        nc.tensor.matmul(
                            pm[:, half * 512:(half + 1) * 512],
                            h3F[:, 2 * kp:2 * kp + 2, mt * P:(mt + 1) * P],
                            w3bS[:, 2 * kp:2 * kp + 2, n * 512:(n + 1) * 512],
                            start=(kp == 0), stop=(kp == KTD // 2 - 1), perf_mode=DR,
                        )
                drain = nc.vector.tensor_copy if mp % 2 == 0 else nc.scalar.copy
                drain(dstage[:, 2 * mp:2 * mp + 2, :],
                      pm[:].rearrange("p (a n) -> p a n", a=2))
            nc.sync.dma_start(
                d["delta"].rearrange("(m p) n -> p m n", p=P)[:, :, n * 512:(n + 1) * 512],
                dstage[:],
            )

    if reps > 1:
        from concourse import mybir as _mb
        stag = int(os.environ.get("KSTAG", "1"))
        kw = {}
        if stag:
            kw = dict(
                staggered_reset=True,
                hint_engines=(
                    _mb.EngineType.PE,
                    _mb.EngineType.Activation,
                    _mb.EngineType.DVE,
                    _mb.EngineType.SP,
                    _mb.EngineType.Pool,
                ),
            )
        with tc.For_i(0, reps, 1, **kw):
            body()
    else:
        body()

    for p in (psX, psW, outp, ftp, bcp, rcp, expp, actp, const):
        p.release()


@functools.lru_cache(maxsize=4)
def _build(reps):
    import concourse.tile as tile
    from concourse import bacc, mybir

    dt = mybir.dt
    nc = bacc.Bacc("TRN2", target_bir_lowering=False, debug=False)
    d = {
        "visF": nc.dram_tensor("visF", [KT1, P, DIM], dt.float8e4, kind="ExternalInput").ap(),
        "bevF": nc.dram_tensor("bevF", [KTB, P, HW], dt.float8e4, kind="ExternalInput").ap(),
        "w1aF": nc.dram_tensor("w1aF", [KT1, P, DIM], dt.float8e4, kind="ExternalInput").ap(),
        "w2aF": nc.dram_tensor("w2aF", [KTB, P, DIM], dt.float8e4, kind="ExternalInput").ap(),
        "w3bF": nc.dram_tensor("w3bF", [KTD, P, HID], dt.float8e4, kind="ExternalInput").ap(),
        "wsmF": nc.dram_tensor("wsmF", [4, KTD, P, DIM], dt.bfloat16, kind="ExternalInput").ap(),
        "wsm8F": nc.dram_tensor("wsm8F", [3, KTD, P, DIM], dt.float8e4, kind="ExternalInput").ap(),
        "wkv8": nc.dram_tensor("wkv8", [2, KTD, P, DIM], dt.float8e4, kind="ExternalInput").ap(),
        "woF": nc.dram_tensor("woF", [HD, NH, KTD, P], dt.float8e4, kind="ExternalInput").ap(),
        "delta": nc.dram_tensor("delta", [NVS, HID], dt.bfloat16, kind="ExternalOutput").ap(),
    }
    with tile.TileContext(nc) as tc:
        _emit(nc, tc, d, reps)
    nc.compile()
    return nc


def _host_prep(inputs):
    hs = np.asarray(inputs["hidden_states"], dtype=np.float32)
    bev = np.asarray(inputs["bev_feat"], dtype=np.float32)
    vis_idx = np.asarray(inputs["vis_idx"])

    def ktile(mat_t, kt):
        # [K, N] -> [kt, 128, N]
        return np.ascontiguousarray(mat_t).reshape(kt, P, -1)

    w1aF = ktile(np.asarray(inputs["w1a"], np.float32).T, KT1).astype(f8)
    w2aF = ktile(np.asarray(inputs["w2a"], np.float32).T, KTB).astype(f8)
    w3bF = ktile(np.asarray(inputs["w3b"], np.float32).T, KTD).astype(f8)
    wsmF = np.stack(
        [
            ktile(np.asarray(inputs[k], np.float32).T, KTD)
            for k in ("w1b", "w2b", "wq", "w3a")
        ]
    ).astype(bf16)
    wsm8F = np.stack(
        [
            ktile(np.asarray(inputs[k], np.float32).T, KTD)
            for k in ("w2b", "wq", "w3a")
        ]
    ).astype(f8)
    wkv8 = np.stack(
        [
            ktile(np.asarray(inputs[k], np.float32).T, KTD)
            for k in ("wk", "wv")
        ]
    ).astype(f8)
    # woF[p, h, dt, m] = Wo[dt*128+m, h*64+p]
    wo = np.asarray(inputs["wo"], np.float32)         # [out, in]
    woF = np.ascontiguousarray(
        wo.T.reshape(NH, HD, KTD, P).transpose(1, 0, 2, 3)
    ).astype(f8)

    vis_by_b = [hs[b][vis_idx[b]] for b in range(B)]  # [NV, HID] f32 each
    in_maps = []
    for c in range(NCORES):
        b, half = c // 2, c % 2
        vis_half = vis_by_b[b][half * NVS:(half + 1) * NVS]
        in_maps.append(
            {
                "visF": ktile(vis_half.T, KT1).astype(f8),
                "bevF": ktile(bev[b].reshape(BEV, HW), KTB).astype(f8),
                "w1aF": w1aF,
                "w2aF": w2aF,
                "w3bF": w3bF,
                "wsmF": wsmF,
                "wsm8F": wsm8F,
                "wkv8": wkv8,
                "woF": woF,
            }
        )
    return hs, vis_idx, vis_by_b, in_maps


def kernel(**inputs):
    from concourse import bass_utils

    nc = _build(REPS)
    hs, vis_idx, vis_by_b, in_maps = _host_prep(inputs)
    res = bass_utils.run_bass_kernel_spmd(nc, in_maps, core_ids=list(range(NCORES)))

    out = hs.copy()
    for c in range(NCORES):
        b, half = c // 2, c % 2
        delta = res.results[c]["delta"].astype(np.float32)
        enh = vis_by_b[b][half * NVS:(half + 1) * NVS] + delta
        out[b][vis_idx[b][half * NVS:(half + 1) * NVS]] = enh
    return out


# revision 27
# speedup vs baseline: 1.1071x; 1.0231x over previous
"""Trainium2 Bass kernel for BEVLayerInjector (8-core SPMD), v3.

Sharding: data-parallel over batch B=4 x 2-way split of the NV=1024 gathered
vision tokens -> 8 shards, one per NeuronCore. The gather (hidden_states ->
vis) and the final scatter + residual add run on the host; the device computes
delta = MLP3(LN(vh + CrossAttn(vh, bh) @ Wo^T)) per shard.

v3 changes vs v2 (which measured ~184us/iter, phase C alone 85us):
  - phase C processes heads in PAIRS: even head on PE rows 0:63, odd head on
    rows 64:127.  Score matmuls for the two heads are emitted interleaved so
    the 64-row subarray groups run concurrently (~2x PE throughput on the
    K=64 score matmuls), and each k-tile's pair of score blocks is drained by
    ONE ACT exp call [128,1024] (ACT is the phase-C bottleneck engine).
  - softmax normalization: per pair ONE DVE reciprocal [1,1024] on the
    ones-rider denominator rows, gpsimd partition_broadcast to 64 rows (was:
    PE broadcast matmul + DVE copy), and ONE DVE multiply for both heads.
    Removes 12 PE matmuls and shortens the per-head cross-engine chain.
  - ao accumulates per pair in one 2-bank psX tile (even head bank a, odd
    head bank b) - no extra PSUM pools.
  - LayerNorm: stats ones are pre-scaled by 1/DIM so mean/E[x^2] come out of
    PSUM directly; var fused into one scalar_tensor_tensor; mean/rstd
    broadcast matmuls write both banks so the final normalize is 4 batched
    DVE ops instead of 8.
  - dead wk/wv copies dropped from the packed bf16 weight bundle (wsm now
    holds only w1b, w2b, wq, w3a).

Layout: "feature-major" tensors keep the contraction dim on SBUF partitions
([feature, token]); weights are shipped pre-transposed as [in, out] and
k-tiled as [ktile, 128, out] so every DMA is contiguous.
"""

import functools
import os
import sys

sys.path.insert(0, "/opt/trn_rl_repo")

import numpy as np
import ml_dtypes

B, S, HID = 4, 4096, 3584
BEV, DIM, NH = 2048, 512, 8
NV, HW, HD = 1024, 1024, 64
EPS = 1e-5
P = 128
NCORES = 8
NVS = NV // 2              # tokens per core (512)
KT1 = HID // P             # 28 k-tiles for HID
KTD = DIM // P             # 4 k-tiles for DIM
KTB = BEV // P             # 16 k-tiles for BEV
MT = NVS // P              # 4 token m-tiles
HWT = HW // P              # 8 HW k-tiles
NO3 = HID // 512           # 7 output column chunks
NPAIR = NH // 2            # 4 head pairs

# index of each 512x512 weight (pre-transposed to [in, out]) in the packed wsm
W1B, W2B, WQ, W3A = range(4)
# fp8 copies for the DoubleRow projections
W2B8, WQ8, W3A8 = range(3)

REPS = 1  # how many times the body is emitted (timing builds use >1)
PHASES = int(os.environ.get("KPHASES", "5"))  # debug: truncate body after N phases
# fp8 DoubleRow knobs for the DIM->DIM projections (1 = fp8-DR, 0 = bf16)
K8BH = int(os.environ.get("K8BH", "1"))
K8Q = int(os.environ.get("K8Q", "1"))
K8MLP3 = int(os.environ.get("K8MLP3", "1"))

bf16 = ml_dtypes.bfloat16
f8 = ml_dtypes.float8_e4m3


def _emit(nc, tc, d, reps):
    from concourse import mybir

    dt = mybir.dt
    AF = mybir.ActivationFunctionType
    DR = mybir.MatmulPerfMode.DoubleRow

    const = tc.alloc_tile_pool(name="const", bufs=1)
    actp = tc.alloc_tile_pool(name="actp", bufs=1)
    expp = tc.alloc_tile_pool(name="expp", bufs=2)
    rcp = tc.alloc_tile_pool(name="rcp", bufs=2)
    bcp = tc.alloc_tile_pool(name="bcp", bufs=2)
    ftp = tc.alloc_tile_pool(name="ftp", bufs=2)
    outp = tc.alloc_tile_pool(name="outp", bufs=2)
    # two rotating pools of 2-bank-wide psum tiles: 2x2x2KB each = all 8 banks
    psW = tc.alloc_tile_pool(name="psW", bufs=2, space="PSUM")
    psX = tc.alloc_tile_pool(name="psX", bufs=2, space="PSUM")

    def wtile(pool):
        tag = "w" if pool is psW else "x"
        return pool.tile([P, 1024], dt.float32, name=tag, tag=tag)

    # ---------------- resident weights (loaded once, before the loop) -------
    w1aS = const.tile([P, KT1, DIM], dt.float8e4, name="w1aS", tag="w1aS")
    w2aS = const.tile([P, KTB, DIM], dt.float8e4, name="w2aS", tag="w2aS")
    w3bS = const.tile([P, KTD, HID], dt.float8e4, name="w3bS", tag="w3bS")
    wsm = const.tile([P, 4, KTD, DIM], dt.bfloat16, name="wsm", tag="wsm")
    wsm8 = const.tile([P, 3, KTD, DIM], dt.float8e4, name="wsm8", tag="wsm8")
    wkv8 = const.tile([P, 2, KTD, DIM], dt.float8e4, name="wkv8", tag="wkv8")
    woF = const.tile([HD, NH, KTD, P], dt.float8e4, name="woF", tag="woF")
    for c in range(2):
        nc.sync.dma_start(
            w1aS[:, c * 14:(c + 1) * 14, :],
            d["w1aF"][c * 14:(c + 1) * 14].rearrange("a p n -> p a n"),
        )
    nc.sync.dma_start(w2aS[:], d["w2aF"].rearrange("a p n -> p a n"))
    nc.sync.dma_start(w3bS[:], d["w3bF"].rearrange("a p n -> p a n"))
    nc.sync.dma_start(wsm[:], d["wsmF"].rearrange("w a p n -> p w a n"))
    nc.sync.dma_start(wsm8[:], d["wsm8F"].rearrange("w a p n -> p w a n"))
    nc.sync.dma_start(wkv8[:], d["wkv8"].rearrange("w a p n -> p w a n"))
    nc.sync.dma_start(woF[:], d["woF"])

    ones_c = const.tile([P, 2], dt.bfloat16, name="ones_c", tag="ones_c")
    nc.vector.memset(ones_c[:], 1.0 / DIM)   # stats matmuls emit mean directly
    ones_r = const.tile([P, P], dt.bfloat16, name="ones_r", tag="ones_r")
    nc.vector.memset(ones_r[:], 1.0)
    eps_t = const.tile([1, 1], dt.float32, name="eps", tag="eps")
    nc.vector.memset(eps_t[:], EPS)

    def body():
        # ---------------- per-iteration activations ------------------------
        visS = actp.tile([P, KT1, DIM], dt.float8e4, name="visS", tag="visS")
        bevS = actp.tile([P, KTB, HW], dt.float8e4, name="bevS", tag="bevS")
        h1T = actp.tile([P, KTD, NVS], dt.bfloat16, name="h1T", tag="h1T")
        h2T = actp.tile([P, KTD, HW], dt.float8e4 if K8BH else dt.bfloat16,
                        name="h2T", tag="h2T")
        vhT = actp.tile([P, KTD, NVS], dt.bfloat16, name="vhT", tag="vhT")
        vhT8 = actp.tile([P, KTD, NVS], dt.float8e4, name="vhT8", tag="vhT8")
        qT = actp.tile([P, KTD, NVS], dt.bfloat16, name="qT", tag="qT")
        bhT = actp.tile([P, KTD, HW], dt.float8e4, name="bhT", tag="bhT")
        kT = actp.tile([P, KTD, HW], dt.bfloat16, name="kT", tag="kT")
        v_ext = actp.tile([P, HWT, NH, 66], dt.float8e4, name="v_ext", tag="v_ext")
        aoT = actp.tile([HD, NH, NVS], dt.float8e4, name="aoT", tag="aoT")
        xT = actp.tile([P, KTD, NVS], dt.bfloat16, name="xT", tag="xT")
        xsq = actp.tile([P, KTD, NVS], dt.bfloat16, name="xsq", tag="xsq")
        # single-partition scratch rows: [0:512] mean, [512:1024] rstd
        rows = actp.tile([1, 2 * NVS], dt.bfloat16, name="rows", tag="rows")
        rtmp = actp.tile([1, 2 * NVS], dt.bfloat16, name="rtmp", tag="rtmp")
        fusedT = actp.tile([P, KTD, NVS], dt.float8e4 if K8MLP3 else dt.bfloat16,
                           name="fusedT", tag="fusedT")
        h3F = actp.tile([P, KTD, NVS], dt.float8e4, name="h3F", tag="h3F")

        nc.vector.memset(v_ext[:, :, :, 64:66], 1.0)

        # ====== phase A: vis MLP L1 and BEV L1, fp8 DoubleRow, interleaved ==
        pmA = [wtile(psW), wtile(psW)]

        def mlp1_step(s):
            if s % 2 == 0:
                c = s // 2  # 7 chunks of 4 k-tiles
                nc.sync.dma_start(
                    visS[:, 4 * c:4 * c + 4, :],
                    d["visF"][4 * c:4 * c + 4].rearrange("a p n -> p a n"),
                )
            for mt in range(KTD):
                nc.tensor.matmul(
                    pmA[mt // 2][:, (mt % 2) * 512:(mt % 2 + 1) * 512],
                    w1aS[:, 2 * s:2 * s + 2, mt * P:(mt + 1) * P],
                    visS[:, 2 * s:2 * s + 2, :],
                    start=(s == 0), stop=(s == 13), perf_mode=DR,
                )
            if s == 13:
                for m in range(2):
                    nc.scalar.activation(h1T[:, 2 * m:2 * m + 2, :], pmA[m][:], AF.Gelu)

        pmB = [None]

        def bev_step(j):
            n, ks = j // 8, j % 8
            if ks == 0:
                pmB[0] = [wtile(psX), wtile(psX)]
            if n == 0 and ks % 2 == 0:
                c = ks // 2  # 4 chunks of 4 k-tiles
                nc.sync.dma_start(
                    bevS[:, 4 * c:4 * c + 4, :],
                    d["bevF"][4 * c:4 * c + 4].rearrange("a p n -> p a n"),
                )
            for mt in range(KTD):
                nc.tensor.matmul(
                    pmB[0][mt // 2][:, (mt % 2) * 512:(mt % 2 + 1) * 512],
                    w2aS[:, 2 * ks:2 * ks + 2, mt * P:(mt + 1) * P],
                    bevS[:, 2 * ks:2 * ks + 2, n * 512:(n + 1) * 512],
                    start=(ks == 0), stop=(ks == 7), perf_mode=DR,
                )
            if ks == 7:
                for m in range(2):
                    nc.scalar.activation(
                        h2T[:, 2 * m:2 * m + 2, n * 512:(n + 1) * 512],
                        pmB[0][m][:].rearrange("p (a n) -> p a n", a=2), AF.Gelu)

        m1_done, bev_done = 0, 0
        for step in range(14 + 16):
            run_m1 = (step < 2 or step % 2 == 0) and m1_done < 14
            if run_m1 or bev_done >= 16:
                mlp1_step(m1_done)
                m1_done += 1
            else:
                bev_step(bev_done)
                bev_done += 1

        if PHASES < 2:
            return
        # ====== phase B: projections (wide psum tiles, paired drains) =======
        def proj_pair(pool, out_ap_fn, stat_fn, moving_fn, drain):
            # two 512-wide outputs accumulated into one wide tile, one drain
            pm = wtile(pool)
            for half in range(2):
                for kt in range(KTD):
                    nc.tensor.matmul(
                        pm[:, half * 512:(half + 1) * 512],
                        stat_fn(half, kt), moving_fn(half, kt),
                        start=(kt == 0), stop=(kt == KTD - 1),
                    )
            drain(out_ap_fn(), pm[:].rearrange("p (a n) -> p a n", a=2))

        # vhT = W1b h1T (dual drains: bf16 for the residual, fp8 for the q-DR)
        for mp in range(2):
            pm = wtile(psW)
            for half in range(2):
                for kt in range(KTD):
                    nc.tensor.matmul(
                        pm[:, half * 512:(half + 1) * 512],
                        wsm[:, W1B, kt, (2 * mp + half) * P:(2 * mp + half + 1) * P],
                        h1T[:, kt, :],
                        start=(kt == 0), stop=(kt == KTD - 1),
                    )
            nc.vector.tensor_copy(vhT[:, 2 * mp:2 * mp + 2, :],
                                  pm[:].rearrange("p (a n) -> p a n", a=2))
            if K8Q:
                with nc.allow_low_precision(reason="fp8 copy feeds q projection only"):
                    nc.gpsimd.tensor_copy(vhT8[:, 2 * mp:2 * mp + 2, :],
                                          vhT[:, 2 * mp:2 * mp + 2, :])

        # bhT = W2b h2T  (DVE drains so ACT can pull the exp table load and
        # the first score exps forward into phase B)
        def bh_half(n):
            for mp in range(2):
                if K8BH:
                    pm = wtile(psX)
                    for half in range(2):
                        mt = 2 * mp + half
                        for kp in range(KTD // 2):
                            nc.tensor.matmul(
                                pm[:, half * 512:(half + 1) * 512],
                                wsm8[:, W2B8, 2 * kp:2 * kp + 2, mt * P:(mt + 1) * P],
                                h2T[:, 2 * kp:2 * kp + 2, n * 512:(n + 1) * 512],
                                start=(kp == 0), stop=(kp == KTD // 2 - 1), perf_mode=DR,
                            )
                    bh_drain = nc.vector.tensor_copy if mp == 0 else nc.scalar.copy
                    bh_drain(
                        bhT[:, 2 * mp:2 * mp + 2, n * 512:(n + 1) * 512],
                        pm[:].rearrange("p (a n) -> p a n", a=2))
                else:
                    proj_pair(
                        psX,
                        lambda mp=mp, n=n: bhT[:, 2 * mp:2 * mp + 2, n * 512:(n + 1) * 512],
                        lambda half, kt, mp=mp: wsm[:, W2B, kt, (2 * mp + half) * P:(2 * mp + half + 1) * P],
                        lambda half, kt, n=n: h2T[:, kt, n * 512:(n + 1) * 512],
                        nc.scalar.copy,
                    )

        bh_half(0)

        # qT = Wq vhT
        for mp in range(2):
            if K8Q:
                pm = wtile(psW)
                for half in range(2):
                    mt = 2 * mp + half
                    for kp in range(KTD // 2):
                        nc.tensor.matmul(
                            pm[:, half * 512:(half + 1) * 512],
                            wsm8[:, WQ8, 2 * kp:2 * kp + 2, mt * P:(mt + 1) * P],
                            vhT8[:, 2 * kp:2 * kp + 2, :],
                            start=(kp == 0), stop=(kp == KTD // 2 - 1), perf_mode=DR,
                        )
                nc.vector.tensor_copy(qT[:, 2 * mp:2 * mp + 2, :],
                                      pm[:].rearrange("p (a n) -> p a n", a=2))
            else:
                proj_pair(
                    psW,
                    lambda mp=mp: qT[:, 2 * mp:2 * mp + 2, :],
                    lambda half, kt, mp=mp: wsm[:, WQ, kt, (2 * mp + half) * P:(2 * mp + half + 1) * P],
                    lambda half, kt: vhT[:, kt, :],
                    nc.vector.tensor_copy,
                )

        bh_half(1)

        # v (token-major, fp8, ones rider col 64) = bhT^T Wv -- before kT so
        # the ao pipeline can start as soon as each head's exp lands
        for q in range(4):
            pm = wtile(psX)
            for half in range(2):
                hw = 2 * q + half
                for kp in range(KTD // 2):
                    nc.tensor.matmul(
                        pm[:, half * 512:(half + 1) * 512],
                        bhT[:, 2 * kp:2 * kp + 2, hw * P:(hw + 1) * P],
                        wkv8[:, 1, 2 * kp:2 * kp + 2, :],
                        start=(kp == 0), stop=(kp == KTD // 2 - 1), perf_mode=DR,
                    )
            v_drain = nc.vector.tensor_copy if q % 2 == 0 else nc.scalar.copy
            v_drain(
                v_ext[:, 2 * q:2 * q + 2, :, 0:64],
                pm[:].rearrange("p (a h e) -> p a h e", a=2, h=NH),
            )

        # kT = Wk bhT (fp8 DoubleRow)
        def k_half(n):
            for mp in range(2):
                pm = wtile(psW)
                for half in range(2):
                    mt = 2 * mp + half
                    for kp in range(KTD // 2):
                        nc.tensor.matmul(
                            pm[:, half * 512:(half + 1) * 512],
                            wkv8[:, 0, 2 * kp:2 * kp + 2, mt * P:(mt + 1) * P],
                            bhT[:, 2 * kp:2 * kp + 2, n * 512:(n + 1) * 512],
                            start=(kp == 0), stop=(kp == KTD // 2 - 1), perf_mode=DR,
                        )
                nc.vector.tensor_copy(
                    kT[:, 2 * mp:2 * mp + 2, n * 512:(n + 1) * 512],
                    pm[:].rearrange("p (a n) -> p a n", a=2))

        k_half(0)
        k_half(1)

        if PHASES < 3:
            return
        # ====== phase C: attention, head PAIRS ==============================
        # pair p: head 2p on PE rows 0:63, head 2p+1 on rows 64:127.
        exp_tiles = {}
        ao_tiles = {}

        def scores_pair(p):
            # expP cols 0:512 = even head, 512:1024 = odd head, per k-tile
            expP = expp.tile([P, HWT, 1024], dt.float8e4, name="expP", tag="expP")
            for kt in range(HWT):
                pm = wtile(psW)
                for par in range(2):  # even head rows 0:64, odd head rows 64:128
                    hp = par * HD
                    nc.tensor.matmul(
                        pm[:, par * 512:(par + 1) * 512],
                        kT[hp:hp + HD, p, kt * P:(kt + 1) * P],
                        qT[hp:hp + HD, p, :],
                        start=True, stop=True,
                    )
                nc.scalar.activation(expP[:, kt, :], pm[:], AF.Exp, scale=0.125)
            exp_tiles[p] = expP

        def ao_pair(p):
            expP = exp_tiles.pop(p)
            pmx = wtile(psX)
            for k in range(HWT // 2):
                for par in range(2):
                    nc.tensor.matmul(
                        pmx[0:65, par * 512:(par + 1) * 512],
                        v_ext[:, 2 * k:2 * k + 2, 2 * p + par, 0:65],
                        expP[:, 2 * k:2 * k + 2, par * 512:(par + 1) * 512],
                        start=(k == 0), stop=(k == HWT // 2 - 1), perf_mode=DR,
                    )
            ao_tiles[p] = pmx

        def tail_pair(p):
            pmx = ao_tiles.pop(p)
            rc = rcp.tile([65, 1024], dt.bfloat16, name="rc", tag="rc")
            with nc.allow_low_precision(reason="softmax denom ~1e3, bf16 recip is plenty"):
                nc.vector.reciprocal(rc[64:65, :], pmx[64:65, :])
            # broadcast the reciprocal row to 64 partitions with an
            # SBUF->SBUF DMA (0-stride partition source): keeps the tail off
            # the PE and out of the PSUM pools so it can't stall the
            # scores->exp rotation.
            bc = bcp.tile([HD, 2, NVS], dt.bfloat16, name="bc", tag="bc")
            nc.sync.dma_start(
                bc[:], rc[64:65, :].unsqueeze(1).broadcast_to((1, HD, 1024)))
            with nc.allow_low_precision(reason="attention out in fp8, tol 2e-2"):
                nc.vector.tensor_mul(
                    aoT[:, 2 * p:2 * p + 2, :],
                    pmx[0:HD, :].rearrange("p (a n) -> p a n", a=2),
                    bc[:])

        KC = int(os.environ.get("KC", "3"))  # debug: 1=scores only, 2=+ao, 3=full
        if KC == 1:
            for p in range(NPAIR):
                scores_pair(p)
                exp_tiles.pop(p)
        elif KC == 2:
            scores_pair(0)
            scores_pair(1)
            ao_pair(0)
            scores_pair(2)
            ao_pair(1)
            scores_pair(3)
            ao_pair(2)
            ao_pair(3)
            for p in range(NPAIR):
                ao_tiles.pop(p)
        else:
            scores_pair(0)
            scores_pair(1)
            ao_pair(0)
            scores_pair(2)
            tail_pair(0)
            ao_pair(1)
            scores_pair(3)
            tail_pair(1)
            ao_pair(2)
            tail_pair(2)
            ao_pair(3)
            tail_pair(3)

        if PHASES < 4:
            return
        # ====== phase D: Wo (feature-major), x = vh + ao Wo^T, LayerNorm ====
        pwo = [wtile(psX), wtile(psX)]
        for dtile in range(KTD):
            for hp2 in range(NH // 2):
                nc.tensor.matmul(
                    pwo[dtile // 2][:, (dtile % 2) * 512:(dtile % 2 + 1) * 512],
                    woF[:, 2 * hp2:2 * hp2 + 2, dtile, :], aoT[:, 2 * hp2:2 * hp2 + 2, :],
                    start=(hp2 == 0), stop=(hp2 == NH // 2 - 1), perf_mode=DR,
                )
        for m in range(2):
            nc.vector.tensor_add(
                xT[:, 2 * m:2 * m + 2, :],
                pwo[m][:].rearrange("p (a n) -> p a n", a=2),
                vhT[:, 2 * m:2 * m + 2, :])
            nc.scalar.activation(xsq[:, 2 * m:2 * m + 2, :], xT[:, 2 * m:2 * m + 2, :], AF.Square)

        # stats (ones are 1/DIM): S1 = mean (bank 0), S2 = E[x^2] (bank 1)
        pst = wtile(psW)
        for kt in range(KTD):
            nc.tensor.matmul(pst[0:1, 0:512], ones_c[:, 0:1], xT[:, kt, :],
                             start=(kt == 0), stop=(kt == KTD - 1))
        for kt in range(KTD):
            nc.tensor.matmul(pst[0:1, 512:1024], ones_c[:, 1:2], xsq[:, kt, :],
                             start=(kt == 0), stop=(kt == KTD - 1))

        # row math, all on partition 0: mean in rows[0:512], rstd in rows[512:]
        with nc.allow_low_precision(reason="LN stats in bf16, tol 2e-2"):
            nc.vector.tensor_copy(rows[0:1, 0:NVS], pst[0:1, 0:512])
            nc.vector.tensor_mul(rtmp[0:1, NVS:], rows[0:1, 0:NVS], rows[0:1, 0:NVS])
            # var = E[x^2]*1 - mean^2, fused psum read + subtract
            nc.vector.scalar_tensor_tensor(
                rtmp[0:1, 0:NVS], pst[0:1, 512:1024], 1.0, rtmp[0:1, NVS:],
                op0=mybir.AluOpType.mult, op1=mybir.AluOpType.subtract)
        nc.scalar.activation(rtmp[0:1, NVS:], rtmp[0:1, 0:NVS], AF.Ln, bias=eps_t[:])
        nc.scalar.activation(rows[0:1, NVS:], rtmp[0:1, NVS:], AF.Exp, scale=-0.5)
        # dummy 1-elem gelu: pulls the ~2.7us natural_log_exp->gelu table
        # switch into ACT's idle stretch here instead of phase E's critical
        # path (no further exp/ln uses this iteration).
        nc.scalar.activation(rtmp[0:1, 2 * NVS - 1:], rtmp[0:1, 2 * NVS - 1:], AF.Gelu)

        # broadcast mean and rstd to all 128 rows, both banks of each tile
        # (psX so phase E's W3b psW rotation isn't gated on the ft reads)
        pmb = wtile(psX)   # mean x2 banks
        pmr = wtile(psX)   # rstd x2 banks
        for bank in range(2):
            nc.tensor.matmul(pmb[:, bank * 512:(bank + 1) * 512],
                             ones_r[0:1, :], rows[0:1, 0:NVS],
                             start=True, stop=True)
            nc.tensor.matmul(pmr[:, bank * 512:(bank + 1) * 512],
                             ones_r[0:1, :], rows[0:1, NVS:],
                             start=True, stop=True)

        for half in range(2):
            ft = ftp.tile([P, 2, NVS], dt.bfloat16, name="ft", tag="ft")
            nc.vector.tensor_sub(
                ft[:], xT[:, 2 * half:2 * half + 2, :],
                pmb[:].rearrange("p (a n) -> p a n", a=2))
            with nc.allow_low_precision(reason="LN output quantized for fp8 MLP3"):
                nc.vector.tensor_mul(
                    fusedT[:, 2 * half:2 * half + 2, :], ft[:],
                    pmr[:].rearrange("p (a n) -> p a n", a=2))

        if PHASES < 5:
            return
        # ====== phase E: output MLP, fp8 DoubleRow for W3b ==================
        for mp in range(2):
            pm = wtile(psX)
            for half in range(2):
                mt = 2 * mp + half
                if K8MLP3:
                    for kp in range(KTD // 2):
                        nc.tensor.matmul(
                            pm[:, half * 512:(half + 1) * 512],
                            wsm8[:, W3A8, 2 * kp:2 * kp + 2, mt * P:(mt + 1) * P],
                            fusedT[:, 2 * kp:2 * kp + 2, :],
                            start=(kp == 0), stop=(kp == KTD // 2 - 1), perf_mode=DR,
                        )
                else:
                    for kt in range(KTD):
                        nc.tensor.matmul(
                            pm[:, half * 512:(half + 1) * 512],
                            wsm[:, W3A, kt, mt * P:(mt + 1) * P], fusedT[:, kt, :],
                            start=(kt == 0), stop=(kt == KTD - 1),
                        )
            nc.scalar.activation(h3F[:, 2 * mp:2 * mp + 2, :], pm[:].rearrange("p (a n) -> p a n", a=2),
                                 AF.Gelu)

        for n in range(NO3):
            dstage = outp.tile([P, MT, 512], dt.bfloat16, name="dstage", tag="dstage")
            for mp in range(2):
                pm = wtile(psW if (n + mp) % 2 == 0 else psX)
                for half in range(2):
                    mt = 2 * mp + half
                    for kp in range(KTD // 2):
                        nc.tensor.matmul(
                            pm[:, half * 512:(half + 1) * 512],
                            h3F[:, 2 * kp:2 * kp + 2, mt * P:(mt + 1) * P],
                            w3bS[:, 2 * kp:2 * kp + 2, n * 512:(n + 1) * 512],
                            start=(kp == 0), stop=(kp == KTD // 2 - 1), perf_mode=DR,
                        )
                drain = nc.vector.tensor_copy if mp % 2 == 0 else nc.scalar.copy
                drain(dstage[:, 2 * mp:2 * mp + 2, :],
                      pm[:].rearrange("p (a n) -> p a n", a=2))
            nc.sync.dma_start(
                d["delta"].rearrange("(m p) n -> p m n", p=P)[:, :, n * 512:(n + 1) * 512],
                dstage[:],
            )

    if reps > 1:
        from concourse import mybir as _mb
        stag = int(os.environ.get("KSTAG", "1"))
        kw = {}
        if stag:
            kw = dict(
                staggered_reset=True,
                hint_engines=(
                    _mb.EngineType.PE,
                    _mb.EngineType.Activation,
                    _mb.EngineType.DVE,
                    _mb.EngineType.SP,
                    _mb.EngineType.Pool,
                ),
            )
        with tc.For_i(0, reps, 1, **kw):
            body()
    else:
        body()

    for p in (psX, psW, outp, ftp, bcp, rcp, expp, actp, const):
        p.release()


@functools.lru_cache(maxsize=4)
def _build(reps):
    import concourse.tile as tile
    from concourse import bacc, mybir

    dt = mybir.dt
    nc = bacc.Bacc("TRN2", target_bir_lowering=False, debug=False)
    d = {
        "visF": nc.dram_tensor("visF", [KT1, P, DIM], dt.float8e4, kind="ExternalInput").ap(),
        "bevF": nc.dram_tensor("bevF", [KTB, P, HW], dt.float8e4, kind="ExternalInput").ap(),
        "w1aF": nc.dram_tensor("w1aF", [KT1, P, DIM], dt.float8e4, kind="ExternalInput").ap(),
        "w2aF": nc.dram_tensor("w2aF", [KTB, P, DIM], dt.float8e4, kind="ExternalInput").ap(),
        "w3bF": nc.dram_tensor("w3bF", [KTD, P, HID], dt.float8e4, kind="ExternalInput").ap(),
        "wsmF": nc.dram_tensor("wsmF", [4, KTD, P, DIM], dt.bfloat16, kind="ExternalInput").ap(),
        "wsm8F": nc.dram_tensor("wsm8F", [3, KTD, P, DIM], dt.float8e4, kind="ExternalInput").ap(),
        "wkv8": nc.dram_tensor("wkv8", [2, KTD, P, DIM], dt.float8e4, kind="ExternalInput").ap(),
        "woF": nc.dram_tensor("woF", [HD, NH, KTD, P], dt.float8e4, kind="ExternalInput").ap(),
        "delta": nc.dram_tensor("delta", [NVS, HID], dt.bfloat16, kind="ExternalOutput").ap(),
    }
    with tile.TileContext(nc) as tc:
        _emit(nc, tc, d, reps)
    nc.compile()
    return nc


def _host_prep(inputs):
    hs = np.asarray(inputs["hidden_states"], dtype=np.float32)
    bev = np.asarray(inputs["bev_feat"], dtype=np.float32)
    vis_idx = np.asarray(inputs["vis_idx"])

    def ktile(mat_t, kt):
        # [K, N] -> [kt, 128, N]
        return np.ascontiguousarray(mat_t).reshape(kt, P, -1)

    w1aF = ktile(np.asarray(inputs["w1a"], np.float32).T, KT1).astype(f8)
    w2aF = ktile(np.asarray(inputs["w2a"], np.float32).T, KTB).astype(f8)
    w3bF = ktile(np.asarray(inputs["w3b"], np.float32).T, KTD).astype(f8)
    wsmF = np.stack(
        [
            ktile(np.asarray(inputs[k], np.float32).T, KTD)
            for k in ("w1b", "w2b", "wq", "w3a")
        ]
    ).astype(bf16)
    wsm8F = np.stack(
        [
            ktile(np.asarray(inputs[k], np.float32).T, KTD)
            for k in ("w2b", "wq", "w3a")
        ]
    ).astype(f8)
    wkv8 = np.stack(
        [
            ktile(np.asarray(inputs[k], np.float32).T, KTD)
            for k in ("wk", "wv")
        ]
    ).astype(f8)
    # woF[p, h, dt, m] = Wo[dt*128+m, h*64+p]
    wo = np.asarray(inputs["wo"], np.float32)         # [out, in]
    woF = np.ascontiguousarray(
        wo.T.reshape(NH, HD, KTD, P).transpose(1, 0, 2, 3)
    ).astype(f8)

    vis_by_b = [hs[b][vis_idx[b]] for b in range(B)]  # [NV, HID] f32 each
    in_maps = []
    for c in range(NCORES):
        b, half = c // 2, c % 2
        vis_half = vis_by_b[b][half * NVS:(half + 1) * NVS]
        in_maps.append(
            {
                "visF": ktile(vis_half.T, KT1).astype(f8),
                "bevF": ktile(bev[b].reshape(BEV, HW), KTB).astype(f8),
                "w1aF": w1aF,
                "w2aF": w2aF,
                "w3bF": w3bF,
                "wsmF": wsmF,
                "wsm8F": wsm8F,
                "wkv8": wkv8,
                "woF": woF,
            }
        )
    return hs, vis_idx, vis_by_b, in_maps


def kernel(**inputs):
    from concourse import bass_utils

    nc = _build(REPS)
    hs, vis_idx, vis_by_b, in_maps = _host_prep(inputs)
    res = bass_utils.run_bass_kernel_spmd(nc, in_maps, core_ids=list(range(NCORES)))

    out = hs.copy()
    for c in range(NCORES):
        b, half = c // 2, c % 2
        delta = res.results[c]["delta"].astype(np.float32)
        enh = vis_by_b[b][half * NVS:(half + 1) * NVS] + delta
        out[b][vis_idx[b][half * NVS:(half + 1) * NVS]] = enh
    return out
